# revision 2
# baseline (speedup 1.0000x reference)
"""Diversity7 loss kernel v4 for Trainium2 (8 NeuronCores, Bass/Tile).

Same math as v3 (self-consistent fp16 pipeline, c=0 chain + chi-corrected
q-pass; see kernel_v3.py docstring). v4 is a scheduling pass:
  - rt0 runs single-model exps so the first bn_stats starts ~5us earlier
    (the first paired exp otherwise waits on two 512KB DMA receipts).
  - a dummy [128,1] activation is issued first so the one-time ACT table
    load (~1.3us) overlaps the initial DMA wait instead of serializing.
  - the ident DMA is deferred behind rt0's x DMAs (needed only at diag(0)).
  - rt3's ln/g is split (models 0-5 / model 6) so its diag+matmul work
    overlaps the final pair's exp+bn instead of serializing into the tail.
  - optional BN3D: one multi-group bn_stats [128,2,500] per model instead
    of two [128,500] calls (needs the bass-level FMAX assert relaxed; the
    HW restriction is per innermost group).
"""

import sys

import numpy as np

if "/opt/trn_rl_repo" not in sys.path:
    sys.path.insert(0, "/opt/trn_rl_repo")

import concourse.bass as bass
import concourse.tile as tile
from concourse import bacc, mybir
from concourse.bass_utils import run_bass_kernel_spmd


def _patch_act_tables() -> None:
    import concourse.hw_specs as hw_specs

    if getattr(hw_specs, "_diversity7_patched", False):
        return
    orig = hw_specs.get_activation_tables

    def patched(module_arch):
        tables = orig(module_arch)
        keep = "natural_log_exp_and_others"
        if keep in tables:
            only = {
                mybir.ActivationFunctionType.Exp,
                mybir.ActivationFunctionType.Ln,
                mybir.ActivationFunctionType.Square,
            }
            for name, funcs in tables.items():
                if name != keep:
                    funcs -= only
        return tables

    hw_specs.get_activation_tables = patched
    bacc.get_activation_tables = patched
    hw_specs._diversity7_patched = True


T = 20.0
SCALE = 0.3
C = 1000
M = 7
N_CORES = 8
ROWS_PER_CORE = 512
RT = ROWS_PER_CORE // 128
ESCALE = 0.7
LNES = float(np.log(ESCALE))
PAIRS = [(0, 1), (2, 3), (4, 5), (6,)]

F32 = mybir.dt.float32
F16 = mybir.dt.float16
AF = mybir.ActivationFunctionType
ALU = mybir.AluOpType

BN3D = False  # walrus birverifier enforces <=512 elems/partition for BNStats


def _build_program() -> bass.Bass:
    _patch_act_tables()
    nc = bacc.Bacc()
    xs = [
        nc.declare_dram_parameter(f"x{m}", [ROWS_PER_CORE, C], F32, isOutput=False)
        for m in range(M)
    ]
    ident_in = nc.declare_dram_parameter("ident", [128, 128], F16, isOutput=False)
    q_out = nc.declare_dram_parameter("q_out", [128, RT], F32, isOutput=True)

    if BN3D:
        nc.vector.BN_STATS_FMAX = 1024  # HW limit is per innermost group

    with tile.TileContext(nc) as tc:
        with (
            tc.tile_pool(name="xp", bufs=6) as xp,
            tc.tile_pool(name="ep", bufs=6) as ep,
            tc.tile_pool(name="bp", bufs=9) as bp,
            tc.tile_pool(name="trp", bufs=2) as trp,
            tc.tile_pool(name="smp", bufs=1) as smp,
            tc.tile_pool(name="psp", bufs=2, space="PSUM") as psp,
            tc.tile_pool(name="qp", bufs=1) as qp,
        ):
            q = qp.tile([128, RT], F32)
            lnb = smp.tile([128, 1], F32, tag="lnb")
            nc.vector.memset(lnb[:], LNES)
            # Dummy tiny activation: pulls the one-time ACT table load to the
            # front so it overlaps the first DMA wait.
            warm = smp.tile([128, 1], F32, tag="warm")
            nc.scalar.activation(warm[:], lnb[:], AF.Exp, bias=0.0, scale=1.0)

            ident = smp.tile([128, 128], F16, tag="ident")

            def bn_model(et, off, stats, m, rt):
                """bn_stats (+aggr) for one model at column offset off in et."""
                if BN3D:
                    bn = bp.tile([128, 2, 6], F32, tag="bn12", name=f"bn_{rt}_{m}")
                    src3 = et[:, off : off + C].rearrange("p (g c) -> p g c", g=2)
                    nc.vector.bn_stats(bn[:], src3)
                else:
                    bn = bp.tile([128, 12], F32, tag="bn12", name=f"bn_{rt}_{m}")
                    nc.vector.bn_stats(bn[:, 0:6], et[:, off : off + 500])
                    nc.vector.bn_stats(bn[:, 6:12], et[:, off + 500 : off + C])
                nc.vector.bn_aggr(stats[:, m, :], bn[:])

            def phase1_rt0():
                """Single-model exps for rt0 — earliest possible DVE start."""
                stats = smp.tile([128, M, 2], F32, tag="stats", bufs=2, name="stats0")
                es = {}
                for m in range(M):
                    xt = xp.tile([128, C], F32, tag="xp1", bufs=4, name=f"x_0_{m}")
                    nc.sync.dma_start(xt[:], xs[m][0:128, :])
                    if m == 1:
                        nc.sync.dma_start(ident[:], ident_in[:])
                    et = ep.tile([128, C], F16, tag="ep1", bufs=8, name=f"e_0_{m}")
                    nc.scalar.activation(et[:], xt[:], AF.Exp, bias=lnb[:],
                                         scale=1.0 / T)
                    bn_model(et, 0, stats, m, 0)
                    es[m] = et[:]
                return stats, es

            def phase1(rt: int, pairs=PAIRS):
                stats = smp.tile([128, M, 2], F32, tag="stats", bufs=2,
                                 name=f"stats{rt}")
                es = {}
                for pi, pair in enumerate(pairs):
                    w = len(pair) * C
                    xt = xp.tile([128, w], F32, tag=f"xp{len(pair)}",
                                 bufs=(6 if len(pair) == 2 else 4),
                                 name=f"x_{rt}_{pi}")
                    for j, m in enumerate(pair):
                        nc.sync.dma_start(
                            xt[:, j * C : (j + 1) * C],
                            xs[m][rt * 128 : (rt + 1) * 128, :],
                        )
                    et = ep.tile([128, w], F16, tag=f"ep{len(pair)}",
                                 bufs=(6 if len(pair) == 2 else 8),
                                 name=f"e_{rt}_{pi}")
                    nc.scalar.activation(et[:], xt[:], AF.Exp, bias=lnb[:],
                                         scale=1.0 / T)
                    for j, m in enumerate(pair):
                        bn_model(et, j * C, stats, m, rt)
                        es[m] = et[:, j * C : (j + 1) * C]
                return stats, es

            def scalars_and_chain(rt, stats, models, gq, ps, first):
                """ln/g/gq/diag/matmul for a subset of models of rt."""
                lo, hi = models[0], models[-1] + 1
                nm = hi - lo
                var_v = stats[:, lo:hi, 1]
                lnr = smp.tile([128, M], F32, tag="lnr", bufs=4,
                               name=f"lnr{rt}_{lo}")
                nc.scalar.activation(lnr[:, 0:nm], var_v, AF.Ln, bias=0.0,
                                     scale=float(C))
                g = smp.tile([128, M], F32, tag="g", bufs=4, name=f"g{rt}_{lo}")
                nc.scalar.activation(g[:, 0:nm], lnr[:, 0:nm], AF.Exp, bias=0.0,
                                     scale=-0.5)
                nc.vector.tensor_copy(gq[:, lo:hi], g[:, 0:nm])
                return g

            def chain(rt, es, g, glo, models, ps):
                for i, m in enumerate(models):
                    dg = trp.tile([128, 128], F16, tag="dg", bufs=3,
                                  name=f"dg{rt}_{m}")
                    nc.vector.tensor_scalar(
                        dg[:], ident[:], g[:, m - glo : m - glo + 1], None,
                        op0=ALU.mult,
                    )
                    nc.tensor.matmul(ps[:, 0:512], dg[:], es[m][:, 0:512],
                                     start=(m == 0), stop=(m == M - 1))
                    nc.tensor.matmul(ps[:, 512:C], dg[:], es[m][:, 512:C],
                                     start=(m == 0), stop=(m == M - 1))

            def qpass(rt, stats, gq, ps):
                mean_v = stats[:, :, 0]
                gm = smp.tile([128, M], F32, tag="gm", bufs=2, name=f"gm{rt}")
                nc.vector.tensor_tensor(gm[:], gq[:], mean_v, ALU.mult)
                negchi = smp.tile([128, 1], F32, tag="negchi", bufs=2,
                                  name=f"negchi{rt}")
                nc.vector.tensor_reduce(negchi[:], gm[:], mybir.AxisListType.X,
                                        ALU.add, negate=True)
                qt = trp.tile([128, C], F16, tag="qt", bufs=2, name=f"qt{rt}")
                nc.scalar.activation(qt[:], ps[:, 0:C], AF.Square, bias=negchi[:],
                                     scale=1.0, accum_out=q[:, rt : rt + 1])

            def phase2(rt, stats, es):
                gq = smp.tile([128, M], F16, tag="gq", bufs=2, name=f"gq{rt}")
                ps = psp.tile([128, 1024], F32, tag="ps", name=f"ps{rt}")
                g = scalars_and_chain(rt, stats, list(range(M)), gq, ps, True)
                chain(rt, es, g, 0, list(range(M)), ps)
                qpass(rt, stats, gq, ps)

            # rt0 + rt1 phase1, then pipelined phase2; rt3 handled with a
            # split tail (models 0-5 chained while model 6 is still in bn).
            s0 = phase1_rt0()
            s1 = phase1(1)
            phase2(0, *s0)
            s2 = phase1(2)
            phase2(1, *s1)
            # rt3: pairs (0,1),(2,3),(4,5) first
            stats3 = smp.tile([128, M, 2], F32, tag="stats", bufs=2, name="stats3")
            es3 = {}
            for pi, pair in enumerate(PAIRS[:3]):
                xt = xp.tile([128, 2 * C], F32, tag="xp2", bufs=6, name=f"x_3_{pi}")
                for j, m in enumerate(pair):
                    nc.sync.dma_start(xt[:, j * C : (j + 1) * C],
                                      xs[m][3 * 128 : 4 * 128, :])
                et = ep.tile([128, 2 * C], F16, tag="ep2", bufs=6, name=f"e_3_{pi}")
                nc.scalar.activation(et[:], xt[:], AF.Exp, bias=lnb[:],
                                     scale=1.0 / T)
                for j, m in enumerate(pair):
                    bn_model(et, j * C, stats3, m, 3)
                    es3[m] = et[:, j * C : (j + 1) * C]
            phase2(2, *s2)
            # rt3 model 6 stream + split scalar tail
            gq3 = smp.tile([128, M], F16, tag="gq", bufs=2, name="gq3")
            ps3 = psp.tile([128, 1024], F32, tag="ps", name="ps3")
            xt6 = xp.tile([128, C], F32, tag="xp1", bufs=4, name="x_3_6")
            nc.sync.dma_start(xt6[:], xs[6][3 * 128 : 4 * 128, :])
            et6 = ep.tile([128, C], F16, tag="ep1", bufs=8, name="e_3_6")
            nc.scalar.activation(et6[:], xt6[:], AF.Exp, bias=lnb[:], scale=1.0 / T)
            ga = scalars_and_chain(3, stats3, list(range(6)), gq3, ps3, True)
            chain(3, es3, ga, 0, list(range(6)), ps3)
            bn_model(et6, 0, stats3, 6, 3)
            es3[6] = et6[:]
            gb = scalars_and_chain(3, stats3, [6], gq3, ps3, False)
            chain(3, es3, gb, 6, [6], ps3)
            qpass(3, stats3, gq3, ps3)
            nc.sync.dma_start(q_out[:], q[:])
    return nc


_NC_CACHE: bass.Bass | None = None


def _get_program() -> bass.Bass:
    global _NC_CACHE
    if _NC_CACHE is None:
        nc = _build_program()
        nc.finalize()
        _NC_CACHE = nc
    return _NC_CACHE


def run_device_part(inputs: dict[str, np.ndarray], **run_kwargs):
    nc = _get_program()
    core_ids = list(range(N_CORES))
    ident = np.eye(128, dtype=np.float16)
    in_maps = []
    for c in range(N_CORES):
        lo, hi = c * ROWS_PER_CORE, (c + 1) * ROWS_PER_CORE
        im = {
            f"x{m}": np.ascontiguousarray(
                inputs[f"outputs{m + 1}"][lo:hi], dtype=np.float32
            )
            for m in range(M)
        }
        im["ident"] = ident
        in_maps.append(im)
    res = run_bass_kernel_spmd(nc, in_maps, core_ids, **run_kwargs)
    qs = []
    for c in range(N_CORES):
        qc = np.asarray(res.results[c]["q_out"])  # [128, RT]
        qs.append(qc.T.reshape(-1))
    q_all = np.concatenate(qs).astype(np.float64)
    return q_all, res


def kernel(**inputs: np.ndarray) -> np.ndarray:
    q_all, _ = run_device_part(inputs)
    loss = SCALE * np.mean((q_all - float(M)) / 2.0)
    return np.float32(loss)


# revision 3
# speedup vs baseline: 1.0521x; 1.0521x over previous
"""Diversity7 loss kernel v4 for Trainium2 (8 NeuronCores, Bass/Tile).

Same math as v3 (self-consistent fp16 pipeline, c=0 chain + chi-corrected
q-pass; see kernel_v3.py docstring). v4 is a scheduling pass:
  - rt0 runs single-model exps so the first bn_stats starts ~5us earlier
    (the first paired exp otherwise waits on two 512KB DMA receipts).
  - a dummy [128,1] activation is issued first so the one-time ACT table
    load (~1.3us) overlaps the initial DMA wait instead of serializing.
  - the ident DMA is deferred behind rt0's x DMAs (needed only at diag(0)).
  - rt3's ln/g is split (models 0-5 / model 6) so its diag+matmul work
    overlaps the final pair's exp+bn instead of serializing into the tail.
  - optional BN3D: one multi-group bn_stats [128,2,500] per model instead
    of two [128,500] calls (needs the bass-level FMAX assert relaxed; the
    HW restriction is per innermost group).
"""

import sys

import numpy as np

if "/opt/trn_rl_repo" not in sys.path:
    sys.path.insert(0, "/opt/trn_rl_repo")

import concourse.bass as bass
import concourse.tile as tile
from concourse import bacc, mybir
from concourse.bass_utils import run_bass_kernel_spmd


def _patch_act_tables() -> None:
    import concourse.hw_specs as hw_specs

    if getattr(hw_specs, "_diversity7_patched", False):
        return
    orig = hw_specs.get_activation_tables

    def patched(module_arch):
        tables = orig(module_arch)
        keep = "natural_log_exp_and_others"
        if keep in tables:
            only = {
                mybir.ActivationFunctionType.Exp,
                mybir.ActivationFunctionType.Ln,
                mybir.ActivationFunctionType.Square,
            }
            for name, funcs in tables.items():
                if name != keep:
                    funcs -= only
        return tables

    hw_specs.get_activation_tables = patched
    bacc.get_activation_tables = patched
    hw_specs._diversity7_patched = True


T = 20.0
SCALE = 0.3
C = 1000
M = 7
N_CORES = 8
ROWS_PER_CORE = 512
RT = ROWS_PER_CORE // 128
ESCALE = 0.7
LNES = float(np.log(ESCALE))
PAIRS = [(0, 1), (2, 3), (4, 5), (6,)]

F32 = mybir.dt.float32
F16 = mybir.dt.float16
AF = mybir.ActivationFunctionType
ALU = mybir.AluOpType

BN3D = False  # walrus birverifier enforces <=512 elems/partition for BNStats


def _build_program() -> bass.Bass:
    _patch_act_tables()
    nc = bacc.Bacc()
    xs = [
        nc.declare_dram_parameter(f"x{m}", [ROWS_PER_CORE, C], F32, isOutput=False)
        for m in range(M)
    ]
    ident_in = nc.declare_dram_parameter("ident", [128, 128], F16, isOutput=False)
    q_out = nc.declare_dram_parameter("q_out", [128, RT], F32, isOutput=True)

    if BN3D:
        nc.vector.BN_STATS_FMAX = 1024  # HW limit is per innermost group

    with tile.TileContext(nc) as tc:
        with (
            tc.tile_pool(name="xp", bufs=6) as xp,
            tc.tile_pool(name="ep", bufs=6) as ep,
            tc.tile_pool(name="bp", bufs=9) as bp,
            tc.tile_pool(name="trp", bufs=2) as trp,
            tc.tile_pool(name="smp", bufs=1) as smp,
            tc.tile_pool(name="psp", bufs=2, space="PSUM") as psp,
            tc.tile_pool(name="qp", bufs=1) as qp,
        ):
            q = qp.tile([128, RT], F32)
            lnb = smp.tile([128, 1], F32, tag="lnb")
            nc.vector.memset(lnb[:], LNES)
            # Dummy tiny activation: pulls the one-time ACT table load to the
            # front so it overlaps the first DMA wait.
            warm = smp.tile([128, 1], F32, tag="warm")
            nc.scalar.activation(warm[:], lnb[:], AF.Exp, bias=0.0, scale=1.0)

            ident = smp.tile([128, 128], F16, tag="ident")

            def bn_model(et, off, stats, m, rt):
                """bn_stats (+aggr) for one model at column offset off in et."""
                if BN3D:
                    bn = bp.tile([128, 2, 6], F32, tag="bn12", name=f"bn_{rt}_{m}")
                    src3 = et[:, off : off + C].rearrange("p (g c) -> p g c", g=2)
                    nc.vector.bn_stats(bn[:], src3)
                else:
                    bn = bp.tile([128, 12], F32, tag="bn12", name=f"bn_{rt}_{m}")
                    nc.vector.bn_stats(bn[:, 0:6], et[:, off : off + 500])
                    nc.vector.bn_stats(bn[:, 6:12], et[:, off + 500 : off + C])
                nc.vector.bn_aggr(stats[:, m, :], bn[:])

            def phase1_rt0():
                """Single-model exps for rt0 — earliest possible DVE start."""
                stats = smp.tile([128, M, 2], F32, tag="stats", bufs=2, name="stats0")
                es = {}
                for m in range(M):
                    xt = xp.tile([128, C], F32, tag="xp1", bufs=4, name=f"x_0_{m}")
                    nc.sync.dma_start(xt[:], xs[m][0:128, :])
                    if m == 1:
                        nc.sync.dma_start(ident[:], ident_in[:])
                    et = ep.tile([128, C], F16, tag="ep1", bufs=8, name=f"e_0_{m}")
                    nc.scalar.activation(et[:], xt[:], AF.Exp, bias=lnb[:],
                                         scale=1.0 / T)
                    bn_model(et, 0, stats, m, 0)
                    es[m] = et[:]
                return stats, es

            def phase1(rt: int, pairs=PAIRS):
                stats = smp.tile([128, M, 2], F32, tag="stats", bufs=2,
                                 name=f"stats{rt}")
                es = {}
                for pi, pair in enumerate(pairs):
                    w = len(pair) * C
                    xt = xp.tile([128, w], F32, tag=f"xp{len(pair)}",
                                 bufs=(6 if len(pair) == 2 else 4),
                                 name=f"x_{rt}_{pi}")
                    for j, m in enumerate(pair):
                        nc.sync.dma_start(
                            xt[:, j * C : (j + 1) * C],
                            xs[m][rt * 128 : (rt + 1) * 128, :],
                        )
                    et = ep.tile([128, w], F16, tag=f"ep{len(pair)}",
                                 bufs=(6 if len(pair) == 2 else 8),
                                 name=f"e_{rt}_{pi}")
                    nc.scalar.activation(et[:], xt[:], AF.Exp, bias=lnb[:],
                                         scale=1.0 / T)
                    for j, m in enumerate(pair):
                        bn_model(et, j * C, stats, m, rt)
                        es[m] = et[:, j * C : (j + 1) * C]
                return stats, es

            def scalars_and_chain(rt, stats, models, gq, ps, first):
                """ln/g/gq/diag/matmul for a subset of models of rt."""
                lo, hi = models[0], models[-1] + 1
                nm = hi - lo
                var_v = stats[:, lo:hi, 1]
                lnr = smp.tile([128, M], F32, tag="lnr", bufs=4,
                               name=f"lnr{rt}_{lo}")
                nc.scalar.activation(lnr[:, 0:nm], var_v, AF.Ln, bias=0.0,
                                     scale=float(C))
                g = smp.tile([128, M], F32, tag="g", bufs=4, name=f"g{rt}_{lo}")
                nc.scalar.activation(g[:, 0:nm], lnr[:, 0:nm], AF.Exp, bias=0.0,
                                     scale=-0.5)
                nc.vector.tensor_copy(gq[:, lo:hi], g[:, 0:nm])
                return g

            def chain(rt, es, g, glo, models, ps):
                for i, m in enumerate(models):
                    dg = trp.tile([128, 128], F16, tag="dg", bufs=4,
                                  name=f"dg{rt}_{m}")
                    gsl = g[:, m - glo : m - glo + 1]
                    if m in (0, 2, 4, 6):
                        nc.scalar.activation(dg[:], ident[:], AF.Copy, bias=0.0,
                                             scale=gsl)
                    else:
                        nc.vector.tensor_scalar(dg[:], ident[:], gsl, None,
                                                op0=ALU.mult)
                    nc.tensor.matmul(ps[:, 0:512], dg[:], es[m][:, 0:512],
                                     start=(m == 0), stop=(m == M - 1))
                    nc.tensor.matmul(ps[:, 512:C], dg[:], es[m][:, 512:C],
                                     start=(m == 0), stop=(m == M - 1))

            def qpass(rt, stats, gq, ps):
                mean_v = stats[:, :, 0]
                gm = smp.tile([128, M], F32, tag="gm", bufs=2, name=f"gm{rt}")
                nc.vector.tensor_tensor(gm[:], gq[:], mean_v, ALU.mult)
                negchi = smp.tile([128, 1], F32, tag="negchi", bufs=2,
                                  name=f"negchi{rt}")
                nc.vector.tensor_reduce(negchi[:], gm[:], mybir.AxisListType.X,
                                        ALU.add, negate=True)
                qt = trp.tile([128, C], F16, tag="qt", bufs=2, name=f"qt{rt}")
                nc.scalar.activation(qt[:], ps[:, 0:C], AF.Square, bias=negchi[:],
                                     scale=1.0, accum_out=q[:, rt : rt + 1])

            def phase2(rt, stats, es):
                gq = smp.tile([128, M], F16, tag="gq", bufs=2, name=f"gq{rt}")
                ps = psp.tile([128, 1024], F32, tag="ps", name=f"ps{rt}")
                g = scalars_and_chain(rt, stats, list(range(M)), gq, ps, True)
                chain(rt, es, g, 0, list(range(M)), ps)
                qpass(rt, stats, gq, ps)

            # rt0 + rt1 phase1, then pipelined phase2; rt3 handled with a
            # split tail (models 0-5 chained while model 6 is still in bn).
            s0 = phase1_rt0()
            s1 = phase1(1)
            phase2(0, *s0)
            s2 = phase1(2)
            phase2(1, *s1)
            # rt3: pairs (0,1),(2,3),(4,5) first
            stats3 = smp.tile([128, M, 2], F32, tag="stats", bufs=2, name="stats3")
            es3 = {}
            for pi, pair in enumerate(PAIRS[:3]):
                xt = xp.tile([128, 2 * C], F32, tag="xp2", bufs=6, name=f"x_3_{pi}")
                for j, m in enumerate(pair):
                    nc.sync.dma_start(xt[:, j * C : (j + 1) * C],
                                      xs[m][3 * 128 : 4 * 128, :])
                et = ep.tile([128, 2 * C], F16, tag="ep2", bufs=6, name=f"e_3_{pi}")
                nc.scalar.activation(et[:], xt[:], AF.Exp, bias=lnb[:],
                                     scale=1.0 / T)
                for j, m in enumerate(pair):
                    bn_model(et, j * C, stats3, m, 3)
                    es3[m] = et[:, j * C : (j + 1) * C]
            phase2(2, *s2)
            # rt3 model 6 stream + split scalar tail
            gq3 = smp.tile([128, M], F16, tag="gq", bufs=2, name="gq3")
            ps3 = psp.tile([128, 1024], F32, tag="ps", name="ps3")
            xt6 = xp.tile([128, C], F32, tag="xp1", bufs=4, name="x_3_6")
            nc.sync.dma_start(xt6[:], xs[6][3 * 128 : 4 * 128, :])
            et6 = ep.tile([128, C], F16, tag="ep1", bufs=8, name="e_3_6")
            nc.scalar.activation(et6[:], xt6[:], AF.Exp, bias=lnb[:], scale=1.0 / T)
            ga = scalars_and_chain(3, stats3, list(range(6)), gq3, ps3, True)
            chain(3, es3, ga, 0, list(range(6)), ps3)
            bn_model(et6, 0, stats3, 6, 3)
            es3[6] = et6[:]
            gb = scalars_and_chain(3, stats3, [6], gq3, ps3, False)
            chain(3, es3, gb, 6, [6], ps3)
            qpass(3, stats3, gq3, ps3)
            nc.sync.dma_start(q_out[:], q[:])
    return nc


_NC_CACHE: bass.Bass | None = None


def _get_program() -> bass.Bass:
    global _NC_CACHE
    if _NC_CACHE is None:
        nc = _build_program()
        nc.finalize()
        _NC_CACHE = nc
    return _NC_CACHE


def run_device_part(inputs: dict[str, np.ndarray], **run_kwargs):
    nc = _get_program()
    core_ids = list(range(N_CORES))
    ident = np.eye(128, dtype=np.float16)
    in_maps = []
    for c in range(N_CORES):
        lo, hi = c * ROWS_PER_CORE, (c + 1) * ROWS_PER_CORE
        im = {
            f"x{m}": np.ascontiguousarray(
                inputs[f"outputs{m + 1}"][lo:hi], dtype=np.float32
            )
            for m in range(M)
        }
        im["ident"] = ident
        in_maps.append(im)
    res = run_bass_kernel_spmd(nc, in_maps, core_ids, **run_kwargs)
    qs = []
    for c in range(N_CORES):
        qc = np.asarray(res.results[c]["q_out"])  # [128, RT]
        qs.append(qc.T.reshape(-1))
    q_all = np.concatenate(qs).astype(np.float64)
    return q_all, res


def kernel(**inputs: np.ndarray) -> np.ndarray:
    q_all, _ = run_device_part(inputs)
    loss = SCALE * np.mean((q_all - float(M)) / 2.0)
    return np.float32(loss)
